# revision 4
# baseline (speedup 1.0000x reference)
"""Interleaved 2x2 upsample kernel for Trainium2 (8 NeuronCores, SPMD).

Input  x: (16, 3, 1024, 1024) f32
Output y: (16, 1, 2048, 2048) f32 where
  y[b, 0, 2i,   2j  ] = x[b, 0, i, j]
  y[b, 0, 2i,   2j+1] = x[b, 1, i, j]
  y[b, 0, 2i+1, 2j  ] = x[b, 2, i, j]
  y[b, 0, 2i+1, 2j+1] = -1

Sharding: pure data parallel over batch (2 batches per core).

Per-core kernel: pure data movement, HBM-bandwidth-bound (56 MiB/core).
Each iteration covers u*128 input rows, partition p holding u consecutive
rows per channel (channel-outer layout -> u*4 KiB contiguous DRAM runs on
the load). Three strided on-chip copies (2x DVE, 1x ACT) build the 2x2
interleave in an output tile where partition p holds 2u consecutive output
rows (u*16 KiB contiguous store runs); constant -1 columns are memset once
per buffer. The u-schedule tapers ([1,1,2,2,2] then [2,2,2,1,1]) so the
first store starts early and the last store has minimal no-overlap tail.
"""

import numpy as np

B, C, H, W = 16, 3, 1024, 1024
N_CORES = 8
B_PER_CORE = B // N_CORES  # 2
P = 128                    # SBUF partitions
UMAX = 2                   # max 128-row units per iteration
NBUF = 3

# per-batch iteration sizes, in 128-row units (must sum to H // P = 8)
SCHED = {0: [1, 1, 2, 2, 2], 1: [2, 2, 2, 1, 1]}

_CACHE = {}


def _build():
    import concourse.bacc as bacc
    import concourse.mybir as mybir
    import concourse.tile as tile

    f32 = mybir.dt.float32
    nc = bacc.Bacc("TRN2", target_bir_lowering=False, debug=False)

    x = nc.dram_tensor("x", [B_PER_CORE, C, H, W], f32, kind="ExternalInput")
    y = nc.dram_tensor("y", [B_PER_CORE, 1, 2 * H, 2 * W], f32, kind="ExternalOutput")

    with tile.TileContext(nc) as tc:
        with tc.tile_pool(name="io", bufs=1) as pool:
            srcs = [
                pool.tile([P, UMAX * C * W], f32, name=f"src{k}", tag=f"src{k}")
                for k in range(NBUF)
            ]
            outs = [
                pool.tile([P, UMAX * 4 * W], f32, name=f"out{k}", tag=f"out{k}")
                for k in range(NBUF)
            ]

            # Constant -1 columns (odd output row, odd output col): written
            # once per buffer, never clobbered. Covers the u=1 prefix too.
            for k in range(NBUF):
                ov = outs[k][:].rearrange(
                    "p (r e j q) -> p r e j q", r=UMAX, e=2, j=W
                )
                nc.gpsimd.memset(ov[:, :, 1, :, 1], -1.0)

            it_idx = 0
            for b in range(B_PER_CORE):
                row0 = 0
                for u in SCHED[b]:
                    k = it_idx % NBUF
                    it_idx += 1
                    src, out = srcs[k], outs[k]

                    # Load: partition p <- rows [row0+u*p, row0+u*(p+1)) of
                    # each channel; channel-outer so each (p, c) run is
                    # u*4096 B contiguous in DRAM.
                    sv = src[:, : u * C * W].rearrange(
                        "p (c r j) -> p c r j", c=C, r=u
                    )
                    xin = x[b][:, row0 : row0 + P * u, :].rearrange(
                        "c (p r) w -> p c r w", r=u
                    )
                    nc.sync.dma_start(out=sv, in_=xin)

                    # Interleave into the output tile: partition p holds
                    # output rows [2*(row0+u*p), 2*(row0+u*p) + 2u).
                    ov = out[:, : u * 4 * W].rearrange(
                        "p (r e j q) -> p r e j q", r=u, e=2, j=W
                    )
                    nc.vector.tensor_copy(ov[:, :, 0, :, 0], sv[:, 0])
                    nc.vector.tensor_copy(ov[:, :, 0, :, 1], sv[:, 1])
                    nc.scalar.copy(ov[:, :, 1, :, 0], sv[:, 2])

                    # Store: u*16 KiB contiguous per partition on both sides.
                    yout = y[b, 0][2 * row0 : 2 * (row0 + P * u), :].rearrange(
                        "(p f) w -> p (f w)", f=2 * u
                    )
                    nc.scalar.dma_start(out=yout, in_=out[:, : u * 4 * W])

                    row0 += P * u

    nc.finalize()
    return nc


def _get_nc():
    if "nc" not in _CACHE:
        _CACHE["nc"] = _build()
    return _CACHE["nc"]


def kernel(x):
    from concourse.bass_utils import run_bass_kernel_spmd

    x = np.ascontiguousarray(np.asarray(x), dtype=np.float32)
    assert x.shape == (B, C, H, W), x.shape

    nc = _get_nc()
    in_maps = [
        {"x": np.ascontiguousarray(x[i * B_PER_CORE : (i + 1) * B_PER_CORE])}
        for i in range(N_CORES)
    ]
    res = run_bass_kernel_spmd(nc, in_maps, list(range(N_CORES))).results
    return np.concatenate([res[i]["y"] for i in range(N_CORES)], axis=0)


# revision 5
# speedup vs baseline: 1.2563x; 1.2563x over previous
"""Interleaved 2x2 upsample kernel for Trainium2 (8 NeuronCores, SPMD).

Input  x: (16, 3, 1024, 1024) f32
Output y: (16, 1, 2048, 2048) f32 where
  y[b, 0, 2i,   2j  ] = x[b, 0, i, j]
  y[b, 0, 2i,   2j+1] = x[b, 1, i, j]
  y[b, 0, 2i+1, 2j  ] = x[b, 2, i, j]
  y[b, 0, 2i+1, 2j+1] = -1

Sharding: pure data parallel over batch (2 batches per core).

Per-core kernel: pure data movement, HBM-bandwidth-bound (56 MiB/core).
Each iteration covers u*128 input rows, partition p holding u consecutive
rows per channel (channel-outer layout -> u*4 KiB contiguous DRAM runs on
the load). Three strided on-chip copies (2x DVE, 1x ACT) build the 2x2
interleave in an output tile where partition p holds 2u consecutive output
rows (u*16 KiB contiguous store runs); constant -1 columns are memset once
per buffer. The u-schedule tapers ([1,1,2,2,2] then [2,2,2,1,1]) so the
first store starts early and the last store has minimal no-overlap tail.
"""

import numpy as np

B, C, H, W = 16, 3, 1024, 1024
N_CORES = 8
B_PER_CORE = B // N_CORES  # 2
P = 128                    # SBUF partitions
UMAX = 2                   # max 128-row units per iteration
NBUF = 3

# per-batch iteration sizes, in 128-row units (must sum to H // P = 8)
SCHED = {0: [2, 2, 2, 2], 1: [2, 2, 2, 2]}

_CACHE = {}


def _build():
    import concourse.bacc as bacc
    import concourse.mybir as mybir
    import concourse.tile as tile

    f32 = mybir.dt.float32
    nc = bacc.Bacc("TRN2", target_bir_lowering=False, debug=False)

    x = nc.dram_tensor("x", [B_PER_CORE, C, H, W], f32, kind="ExternalInput")
    y = nc.dram_tensor("y", [B_PER_CORE, 1, 2 * H, 2 * W], f32, kind="ExternalOutput")

    with tile.TileContext(nc) as tc:
        with tc.tile_pool(name="io", bufs=1) as pool:
            srcs = [
                pool.tile([P, UMAX * C * W], f32, name=f"src{k}", tag=f"src{k}")
                for k in range(NBUF)
            ]
            outs = [
                pool.tile([P, UMAX * 4 * W], f32, name=f"out{k}", tag=f"out{k}")
                for k in range(NBUF)
            ]

            # Constant -1 columns (odd output row, odd output col): written
            # once per buffer, never clobbered. Covers the u=1 prefix too.
            for k in range(NBUF):
                ov = outs[k][:].rearrange(
                    "p (r e j q) -> p r e j q", r=UMAX, e=2, j=W
                )
                nc.gpsimd.memset(ov[:, :, 1, :, 1], -1.0)

            it_idx = 0
            for b in range(B_PER_CORE):
                row0 = 0
                for u in SCHED[b]:
                    k = it_idx % NBUF
                    it_idx += 1
                    src, out = srcs[k], outs[k]

                    # Load: partition p <- rows [row0+u*p, row0+u*(p+1)) of
                    # each channel; channel-outer so each (p, c) run is
                    # u*4096 B contiguous in DRAM.
                    sv = src[:, : u * C * W].rearrange(
                        "p (c r j) -> p c r j", c=C, r=u
                    )
                    xin = x[b][:, row0 : row0 + P * u, :].rearrange(
                        "c (p r) w -> p c r w", r=u
                    )
                    nc.sync.dma_start(out=sv, in_=xin)

                    # Interleave into the output tile: partition p holds
                    # output rows [2*(row0+u*p), 2*(row0+u*p) + 2u).
                    ov = out[:, : u * 4 * W].rearrange(
                        "p (r e j q) -> p r e j q", r=u, e=2, j=W
                    )
                    nc.vector.tensor_copy(ov[:, :, 0, :, 0], sv[:, 0])
                    nc.vector.tensor_copy(ov[:, :, 0, :, 1], sv[:, 1])
                    nc.vector.tensor_copy(ov[:, :, 1, :, 0], sv[:, 2])

                    # Store: u*16 KiB contiguous per partition on both sides.
                    yout = y[b, 0][2 * row0 : 2 * (row0 + P * u), :].rearrange(
                        "(p f) w -> p (f w)", f=2 * u
                    )
                    nc.scalar.dma_start(out=yout, in_=out[:, : u * 4 * W])

                    row0 += P * u

    nc.finalize()
    return nc


def _get_nc():
    if "nc" not in _CACHE:
        _CACHE["nc"] = _build()
    return _CACHE["nc"]


def kernel(x):
    from concourse.bass_utils import run_bass_kernel_spmd

    x = np.ascontiguousarray(np.asarray(x), dtype=np.float32)
    assert x.shape == (B, C, H, W), x.shape

    nc = _get_nc()
    in_maps = [
        {"x": np.ascontiguousarray(x[i * B_PER_CORE : (i + 1) * B_PER_CORE])}
        for i in range(N_CORES)
    ]
    res = run_bass_kernel_spmd(nc, in_maps, list(range(N_CORES))).results
    return np.concatenate([res[i]["y"] for i in range(N_CORES)], axis=0)
